# revision 1
# baseline (speedup 1.0000x reference)
"""Trainium2 Bass kernel for a GRU actor-critic network.

Reference computation (per batch row b of B=4096):
    x_gates[t] = features[b,t,:] @ w_ih.T + b_ih            # [T, 3H]
    GRU scan over T=64 steps (torch gate order r, z, n):
        r = sigmoid(xr + hr + b_ihr + b_hhr)
        z = sigmoid(xz + hz + b_ihz + b_hhz)
        n = tanh(xn + b_ihn + r * (hn + b_hhn))
        h = (1-z)*n + z*h
    out = leaky_relu(h_last)
    pi  = leaky_relu(out @ w_pi.T + b_pi)                   # [B, 64]
    vf  = leaky_relu(out @ w_vf.T + b_vf)                   # [B, 64]

Strategy: pure data parallel over 8 cores (512 batch rows each).  On-chip
layout is [gate/hidden on partitions, batch on free] so the recurrent
matmul contracts over the partition dim without per-step transposes.

This version (v3) restructures the per-step pipeline around the serial
dependency chain (the baseline was latency-bound at ~9us/step):
  * 4 independent batch chains of 128 columns each, staggered in time, so
    each chain's serial chain hides behind the other chains' engine work
    and every elementwise instruction is [128, 256-512] (small latency).
  * Recurrent GEMM in fp8 e4m3 DoubleRow mode (K=256 per instruction,
    0.5 cycles/col) - 4x fewer PE cycles than bf16.  h is kept in bf16
    for the elementwise update (precision) and written a second time as
    fp8 for the next matmul (emulated end-to-end rel err ~0.008).
  * All gate biases are pre-merged into PSUM by tiny K=4 one-hot bf16
    matmuls that open each accumulation group, so sigmoid over the whole
    [r0 r1 z0 z1] bank is ONE activation instruction and tr/u are plain
    tensor_tensor ops (no per-m-tile bias splits).
  * Elementwise placement: DVE does the PSUM-sourced ops (tr, u) + d, e;
    GPSIMD (Pool) does the two h writes (bf16 + fp8 copies of n+e).
"""

import os
import sys

import numpy as np
import ml_dtypes

if "/opt/trn_rl_repo" not in sys.path:
    sys.path.insert(0, "/opt/trn_rl_repo")

P = 128          # partitions
H = 256          # GRU hidden
F = 128          # feature dim
T = 64           # sequence length
OUT = 64         # head dim
B = 4096         # full batch
NCORES = 8
BLOC = B // NCORES   # 512 rows per core
CH = 2               # independent batch chains per core
BC = BLOC // CH      # 256 rows per chain
NEG_SLOPE = 0.01

_cache = {}


def build_nc(t_steps=T, loop_n=1):
    import concourse.bass as bass
    import concourse.tile as tile
    from concourse import bacc, mybir

    f32 = mybir.dt.float32
    bf16 = mybir.dt.bfloat16
    fp8 = mybir.dt.float8e4
    AF = mybir.ActivationFunctionType
    OP = mybir.AluOpType
    PSUM = bass.MemorySpace.PSUM
    DR = mybir.MatmulPerfMode.DoubleRow

    nc = bacc.Bacc("TRN2", target_bir_lowering=False, debug=False)

    featT = nc.declare_dram_parameter("featT", [T, F, BLOC], bf16, isOutput=False)
    w_ihT = nc.declare_dram_parameter("w_ihT", [P, 6 * P], bf16, isOutput=False)
    w_hh8 = nc.declare_dram_parameter("w_hh8", [P, 2, 6 * P], fp8, isOutput=False)
    # bias lhsT for the 4 PSUM banks: [bank(rza,rzb,xha,xhb), row, ksub, P]
    bias4 = nc.declare_dram_parameter("bias4", [4, 2, 2, P], fp8, isOutput=False)
    onehot = nc.declare_dram_parameter("onehot", [2, 2, 2 * BC], fp8, isOutput=False)
    w_piT = nc.declare_dram_parameter("w_piT", [P, 2, OUT], bf16, isOutput=False)
    w_vfT = nc.declare_dram_parameter("w_vfT", [P, 2, OUT], bf16, isOutput=False)
    b_pv = nc.declare_dram_parameter("b_pv", [P, 2, OUT], f32, isOutput=False)
    out_pi = nc.declare_dram_parameter("pi", [BLOC, OUT], f32, isOutput=True)
    out_vf = nc.declare_dram_parameter("vf", [BLOC, OUT], f32, isOutput=True)

    with tile.TileContext(nc) as tc:
        from contextlib import ExitStack

        ctx = ExitStack()
        with ctx:
            singles = ctx.enter_context(tc.tile_pool(name="singles", bufs=1))
            hsb = ctx.enter_context(tc.tile_pool(name="hsb", bufs=4))

            # ---- weights / biases ----
            sb_wih = singles.tile([P, 6 * P], bf16)
            nc.sync.dma_start(out=sb_wih, in_=w_ihT[:, :])
            sb_whh8 = singles.tile([P, 2, 6 * P], fp8)
            nc.sync.dma_start(out=sb_whh8, in_=w_hh8[:, :, :])
            sb_bias = []
            for bk in range(4):
                tb = singles.tile([2, 2, P], fp8, tag=f"bias{bk}")
                nc.sync.dma_start(out=tb, in_=bias4[bk, :, :, :])
                sb_bias.append(tb)
            sb_oh = singles.tile([2, 2, 2 * BC], fp8)
            nc.sync.dma_start(out=sb_oh, in_=onehot[:, :, :])
            sb_wpi = singles.tile([P, 2, OUT], bf16)
            nc.sync.dma_start(out=sb_wpi, in_=w_piT[:, :, :])
            sb_wvf = singles.tile([P, 2, OUT], bf16)
            nc.sync.dma_start(out=sb_wvf, in_=w_vfT[:, :, :])
            sb_bpv = singles.tile([P, 2, OUT], f32)
            nc.sync.dma_start(out=sb_bpv, in_=b_pv[:, :, :])

            # ---- features: host-pre-transposed [t, f, b] bf16, direct DMA ----
            fT = singles.tile([P, t_steps, BLOC], bf16)
            n_chunk_t = min(8, t_steps)
            for c in range(t_steps // n_chunk_t):
                sl = slice(c * n_chunk_t, (c + 1) * n_chunk_t)
                nc.sync.dma_start(
                    out=fT[:, sl, :],
                    in_=featT[sl, :, :].rearrange("t f b -> f t b"),
                )

            # ---- recurrence ----
            loop_ctx = ExitStack()
            if loop_n > 1:
                loop_ctx.enter_context(tc.For_i(0, loop_n, 1))
            with loop_ctx, ExitStack() as rctx:
                ps_rza = [
                    rctx.enter_context(
                        tc.tile_pool(name=f"ps_rza{c}", bufs=1, space=PSUM)
                    )
                    for c in range(CH)
                ]
                ps_rzb = [
                    rctx.enter_context(
                        tc.tile_pool(name=f"ps_rzb{c}", bufs=1, space=PSUM)
                    )
                    for c in range(CH)
                ]
                ps_xha = [
                    rctx.enter_context(
                        tc.tile_pool(name=f"ps_xha{c}", bufs=1, space=PSUM)
                    )
                    for c in range(CH)
                ]
                ps_xhb = [
                    rctx.enter_context(
                        tc.tile_pool(name=f"ps_xhb{c}", bufs=1, space=PSUM)
                    )
                    for c in range(CH)
                ]
                gates = [
                    rctx.enter_context(tc.tile_pool(name=f"gates{c}", bufs=2))
                    for c in range(CH)
                ]
                hpool = [
                    rctx.enter_context(tc.tile_pool(name=f"hpool{c}", bufs=2))
                    for c in range(CH)
                ]
                h8pool = [
                    rctx.enter_context(tc.tile_pool(name=f"h8pool{c}", bufs=2))
                    for c in range(CH)
                ]

                h_prev = []
                h8_prev = []
                for c in range(CH):
                    h0 = hpool[c].tile([P, 2 * BC], bf16, tag="h")
                    nc.vector.memset(h0, 0.0)
                    h_prev.append(h0)
                    h80 = h8pool[c].tile([P, 2, BC], fp8, tag="h8")
                    nc.gpsimd.memset(h80, 0.0)
                    h8_prev.append(h80)

                for t in range(t_steps):
                    rza_t = [None] * CH
                    rzb_t = [None] * CH
                    xha_t = [None] * CH
                    xhb_t = [None] * CH
                    for pair in ((0, 1),):
                        # --- allocate this step's PSUM tiles (1 bank each) ---
                        for c in pair:
                            rza_t[c] = ps_rza[c].tile(
                                [P, 2 * BC], f32, tag="rza", name="rza"
                            )
                            rzb_t[c] = ps_rzb[c].tile(
                                [P, 2 * BC], f32, tag="rzb", name="rzb"
                            )
                            xha_t[c] = ps_xha[c].tile(
                                [P, 2 * BC], f32, tag="xha", name="xha"
                            )
                            xhb_t[c] = ps_xhb[c].tile(
                                [P, 2 * BC], f32, tag="xhb", name="xhb"
                            )
                        # --- bias pre-merge matmuls open the accum groups
                        # (fp8 DoubleRow: 2nd k-subtile is zeros, 0.5 cyc/col)
                        for bk, tiles in enumerate(
                            (rza_t, rzb_t, xha_t, xhb_t)
                        ):
                            for c in pair:
                                nc.tensor.matmul(
                                    tiles[c],
                                    sb_bias[bk],
                                    sb_oh,
                                    start=True,
                                    stop=False,
                                    perf_mode=DR,
                                )
                        # --- input GEMMs, grouped by stationary weight ---
                        for g in range(4):  # r0 r1 z0 z1
                            tiles = rza_t if g < 2 else rzb_t
                            blk = g % 2
                            for c in pair:
                                nc.tensor.matmul(
                                    tiles[c][:, blk * BC : (blk + 1) * BC],
                                    sb_wih[:, g * P : (g + 1) * P],
                                    fT[:, t, c * BC : (c + 1) * BC],
                                    start=False,
                                    stop=False,
                                )
                        for g in range(2):  # xn m-tiles
                            for c in pair:
                                nc.tensor.matmul(
                                    xha_t[c][:, g * BC : (g + 1) * BC],
                                    sb_wih[:, (4 + g) * P : (5 + g) * P],
                                    fT[:, t, c * BC : (c + 1) * BC],
                                    start=False,
                                    stop=(g == 1),
                                )
                        # --- recurrent GEMMs: fp8 DoubleRow, K=256/instr ---
                        for g in range(4):  # rz
                            tiles = rza_t if g < 2 else rzb_t
                            blk = g % 2
                            for c in pair:
                                nc.tensor.matmul(
                                    tiles[c][:, blk * BC : (blk + 1) * BC],
                                    sb_whh8[:, :, g * P : (g + 1) * P],
                                    h8_prev[c][:, :, :],
                                    start=False,
                                    stop=(blk == 1),
                                    perf_mode=DR,
                                )
                        for g in range(2):  # hn
                            for c in pair:
                                nc.tensor.matmul(
                                    xhb_t[c][:, g * BC : (g + 1) * BC],
                                    sb_whh8[:, :, (4 + g) * P : (5 + g) * P],
                                    h8_prev[c][:, :, :],
                                    start=False,
                                    stop=(g == 1),
                                    perf_mode=DR,
                                )

                    for c in range(CH):
                        hp = h_prev[c]
                        xn = xha_t[c]
                        hn = xhb_t[c]

                        r_s = gates[c].tile([P, 2 * BC], bf16, tag="r_s")
                        z_s = gates[c].tile([P, 2 * BC], bf16, tag="z_s")
                        trt = gates[c].tile([P, 2 * BC], bf16, tag="tr")
                        u = gates[c].tile([P, 2 * BC], bf16, tag="u")
                        nt = gates[c].tile([P, 2 * BC], bf16, tag="nt")
                        d = gates[c].tile([P, 2 * BC], bf16, tag="d")
                        e = gates[c].tile([P, 2 * BC], bf16, tag="e")

                        # split sigmoid: r first (on the tr chain), z separate
                        nc.scalar.activation(r_s, rza_t[c], AF.Sigmoid)
                        nc.scalar.activation(z_s, rzb_t[c], AF.Sigmoid)
                        # tr = (hn + b_hhn) * r   (bias premerged in PSUM)
                        nc.vector.tensor_tensor(trt, hn, r_s, OP.mult)
                        # u = (xn + b_ihn) + tr
                        nc.vector.tensor_tensor(u, xn, trt, OP.add)
                        nc.scalar.activation(nt, u, AF.Tanh)
                        # h' = n + z*(h - n)
                        nc.vector.tensor_tensor(d, hp, nt, OP.subtract)
                        nc.vector.tensor_tensor(e, z_s, d, OP.mult)
                        h8_new = h8pool[c].tile([P, 2, BC], fp8, tag="h8")
                        nc.vector.tensor_tensor(
                            h8_new.rearrange("p j c -> p (j c)"), nt, e, OP.add
                        )
                        h_new = hpool[c].tile([P, 2 * BC], bf16, tag="h")
                        nc.gpsimd.tensor_tensor(h_new, nt, e, OP.add)
                        h_prev[c] = h_new
                        h8_prev[c] = h8_new

            # ---- heads ----
            with ExitStack() as hctx:
                pshead = hctx.enter_context(
                    tc.tile_pool(name="pshead", bufs=4, space=PSUM)
                )
                for c in range(CH):
                    lt = singles.tile([P, 2 * BC], bf16, tag=f"lr{c}")
                    nc.vector.scalar_tensor_tensor(
                        out=lt,
                        in0=h_prev[c],
                        scalar=NEG_SLOPE,
                        in1=h_prev[c],
                        op0=OP.mult,
                        op1=OP.max,
                    )
                    for head, (wT, out_dram) in enumerate(
                        [(sb_wpi, out_pi), (sb_wvf, out_vf)]
                    ):
                        for m in range(BC // P):
                            pp = pshead.tile([P, OUT], f32, tag="pp")
                            for j in range(2):
                                nc.tensor.matmul(
                                    pp,
                                    lt[:, j * BC + m * P : j * BC + (m + 1) * P],
                                    wT[:, j, :],
                                    start=(j == 0),
                                    stop=(j == 1),
                                )
                            q = hsb.tile([P, OUT], f32, tag="q")
                            nc.vector.tensor_tensor(
                                q, pp, sb_bpv[:, head, :], OP.add
                            )
                            o = hsb.tile([P, OUT], f32, tag="o")
                            nc.vector.scalar_tensor_tensor(
                                out=o,
                                in0=q,
                                scalar=NEG_SLOPE,
                                in1=q,
                                op0=OP.mult,
                                op1=OP.max,
                            )
                            r0 = c * BC + m * P
                            nc.scalar.dma_start(
                                out=out_dram[r0 : r0 + P, :], in_=o
                            )

    return nc


def prep_inputs(inputs):
    """Host-side prep: shard features, build weight/bias layouts."""
    bf = ml_dtypes.bfloat16
    e4 = ml_dtypes.float8_e4m3
    feat = np.asarray(inputs["features"], np.float32).reshape(B, T, F)
    w_ih = np.asarray(inputs["w_ih"], np.float32)
    w_hh = np.asarray(inputs["w_hh"], np.float32)
    b_ih = np.asarray(inputs["b_ih"], np.float32)
    b_hh = np.asarray(inputs["b_hh"], np.float32)
    w_pi = np.asarray(inputs["w_pi"], np.float32)
    b_pi = np.asarray(inputs["b_pi"], np.float32)
    w_vf = np.asarray(inputs["w_vf"], np.float32)
    b_vf = np.asarray(inputs["b_vf"], np.float32)

    w_ihT = np.ascontiguousarray(w_ih.T).astype(bf)                       # [128, 768]
    w_hh8 = np.ascontiguousarray(
        w_hh.T.reshape(2, P, 6 * P).transpose(1, 0, 2)
    ).astype(e4)                                                          # [128, 2, 768]
    b_c = b_ih + b_hh
    # bias rows per PSUM bank: rza=[r0,r1], rzb=[z0,z1], xha=[ihn0,ihn1],
    # xhb=[hhn0,hhn1]; second k-subtile is zeros.
    bias4 = np.zeros((4, 2, 2, P), np.float32)
    bias4[0, :, 0, :] = [b_c[0:128], b_c[128:256]]
    bias4[1, :, 0, :] = [b_c[256:384], b_c[384:512]]
    bias4[2, :, 0, :] = [b_ih[512:640], b_ih[640:768]]
    bias4[3, :, 0, :] = [b_hh[512:640], b_hh[640:768]]
    bias4 = bias4.astype(e4)
    onehot = np.zeros((2, 2, 2 * BC), np.float32)
    for j in range(2):
        onehot[j, 0, j * BC : (j + 1) * BC] = 1.0
    onehot = onehot.astype(e4)                                            # [2, 2, 512]

    w_piT = np.ascontiguousarray(
        w_pi.T.reshape(2, P, OUT).transpose(1, 0, 2)
    ).astype(bf)
    w_vfT = np.ascontiguousarray(
        w_vf.T.reshape(2, P, OUT).transpose(1, 0, 2)
    ).astype(bf)
    b_pv = np.ascontiguousarray(
        np.broadcast_to(np.stack([b_pi, b_vf], axis=0), (P, 2, OUT))
    ).astype(np.float32)

    shared = {
        "w_ihT": w_ihT,
        "w_hh8": w_hh8,
        "bias4": bias4,
        "onehot": onehot,
        "w_piT": w_piT,
        "w_vfT": w_vfT,
        "b_pv": b_pv,
    }
    in_maps = []
    for i in range(NCORES):
        m = dict(shared)
        shard = feat[i * BLOC : (i + 1) * BLOC]        # [BLOC, T, F]
        m["featT"] = np.ascontiguousarray(
            shard.transpose(1, 2, 0)
        ).astype(bf)                                    # [T, F, BLOC]
        in_maps.append(m)
    return in_maps


def _get_nc():
    if "nc" not in _cache:
        nc = build_nc()
        nc.finalize()
        _cache["nc"] = nc
    return _cache["nc"]


def _get_runner():
    """Build (once) a cached jitted shard_map executor for the bass program."""
    if "runner" in _cache:
        return _cache["runner"]

    import jax
    from jax.experimental.shard_map import shard_map
    from jax.sharding import Mesh, PartitionSpec
    from concourse import bass2jax, mybir

    nc = _get_nc()
    bass2jax.install_neuronx_cc_hook()

    partition_name = (
        nc.partition_id_tensor.name if nc.partition_id_tensor else None
    )
    in_names, out_names, out_avals, zero_outs = [], [], [], []
    for alloc in nc.m.functions[0].allocations:
        if not isinstance(alloc, mybir.MemoryLocationSet):
            continue
        name = alloc.memorylocations[0].name
        if alloc.kind == "ExternalInput":
            if name != partition_name:
                in_names.append(name)
        elif alloc.kind == "ExternalOutput":
            out_names.append(name)
            shape = tuple(alloc.tensor_shape)
            dtype = mybir.dt.np(alloc.dtype)
            out_avals.append(jax.core.ShapedArray(shape, dtype))
            zero_outs.append(np.zeros(shape, dtype))
    n_params = len(in_names)
    n_outs = len(out_avals)
    all_names = in_names + out_names
    if partition_name is not None:
        all_names = all_names + [partition_name]

    def _body(*args):
        operands = list(args)
        if partition_name is not None:
            operands.append(bass2jax.partition_id_tensor())
        outs = bass2jax._bass_exec_p.bind(
            *operands,
            out_avals=tuple(out_avals),
            in_names=tuple(all_names),
            out_names=tuple(out_names),
            lowering_input_output_aliases=(),
            sim_require_finite=True,
            sim_require_nnan=True,
            nc=nc,
        )
        return tuple(outs)

    donate = tuple(range(n_params, n_params + n_outs))
    devices = jax.devices()[:NCORES]
    mesh = Mesh(np.asarray(devices), ("core",))
    sharded = jax.jit(
        shard_map(
            _body,
            mesh=mesh,
            in_specs=(PartitionSpec("core"),) * (n_params + n_outs),
            out_specs=(PartitionSpec("core"),) * n_outs,
            check_rep=False,
        ),
        donate_argnums=donate,
        keep_unused=True,
    )

    from jax.sharding import NamedSharding

    shard_spec = NamedSharding(mesh, PartitionSpec("core"))
    state = {}

    def run(in_maps, timeit=False):
        key = id(in_maps)
        if state.get("key") != key:
            concat_in = [
                np.concatenate([np.asarray(m[n]) for m in in_maps], axis=0)
                for n in in_names
            ]
            state["dev_in"] = [
                jax.device_put(a, shard_spec) for a in concat_in
            ]
            for a in state["dev_in"]:
                a.block_until_ready()
            state["key"] = key
        concat_zeros = [
            jax.device_put(
                np.zeros((NCORES * z.shape[0], *z.shape[1:]), z.dtype),
                shard_spec,
            )
            for z in zero_outs
        ]
        out_arrs = sharded(*state["dev_in"], *concat_zeros)
        jax.block_until_ready(out_arrs)
        outs = {
            name: np.asarray(out_arrs[i]) for i, name in enumerate(out_names)
        }
        return outs

    _cache["runner"] = run
    return run


def kernel(**inputs):
    run = _get_runner()
    in_maps = prep_inputs(inputs)
    outs = run(in_maps)
    pi = outs["pi"].astype(np.float32)
    vf = outs["vf"].astype(np.float32)
    return pi, vf


def kernel_timed(inputs, iters=10):
    """Returns (pi, vf, per_call_seconds) with device-resident inputs."""
    import time

    run = _get_runner()
    in_maps = prep_inputs(inputs)
    outs = run(in_maps)  # warmup + input upload
    t0 = time.monotonic()
    for _ in range(iters):
        outs = run(in_maps)
    dt = (time.monotonic() - t0) / iters
    pi = outs["pi"].astype(np.float32)
    vf = outs["vf"].astype(np.float32)
    return pi, vf, dt

